# revision 5
# baseline (speedup 1.0000x reference)
"""3-layer GAT (PPI-style) forward on 8 Trainium2 NeuronCores — v2.

Strategy (SPMD, one NEFF on 8 cores):
  - Host: add self-loops, degree-balanced node permutation into 8 cores x
    2500 nodes (tiles of 128 dst nodes), edges sorted by dst and padded to a
    uniform chunk count; int16 gather-index arrays and STATIC one-hot
    scatter matrices (oh: edges->dst rows, ohT: transpose) precomputed.
  - Dense phase per tile: [h | lin] = x @ [W | Wl] (bf16, PE); es/ed
    attention dots folded into the matmul as extra columns (waSD = [W@a_s |
    W@a_d]); payload row [h0|1|h1|1|...|es] (bf16 + f32 es tail) staged to
    DRAM shard.
  - AllGather payload across cores.
  - Aggregation per tile: ed via tiny matmul ohT^T @ edt; one payload
    dma_gather per group; w = exp(leaky(es+ed)) scales the payload in-place
    with a single broadcast multiply; scatter-sum via matmul with the static
    one-hot lhsT batched over all heads (512-col chunks, PSUM-accumulated);
    normalize, add skip+bias, ELU; PE-transpose keeps the next layer's lhsT
    in SBUF (no DRAM roundtrip, no DMA transposes); next layer's dense phase
    is inlined per tile for overlap.
"""

import math
import numpy as np

N_CORES = 8
GROUP = 6  # gather chunks per dma_gather group
DEBUG_PROBES = False  # insert sim/hw divergence probes (li=1, t=0)
SPLIT_AG = False  # two half-shard AllGathers per layer; blocked by the
                  # framework's single-writer check on Shared DRAM tensors


# --------------------------------------------------------------------------
# host-side prep (data layout / graph partitioning / static one-hots)
# --------------------------------------------------------------------------

def _balance_permutation(dst, n, n_cores, tiles_per_core, rows_last):
    """Greedy balance: nodes -> 128-row dst tiles with ~equal edge counts."""
    import heapq

    deg = np.bincount(dst, minlength=n).astype(np.int64)
    order = np.argsort(-deg, kind="stable")
    n_tiles = n_cores * tiles_per_core
    caps = np.full(n_tiles, 128, np.int64)
    caps[tiles_per_core - 1 :: tiles_per_core] = rows_last
    heap = [(0, int(b)) for b in range(n_tiles)]
    heapq.heapify(heap)
    members = [[] for _ in range(n_tiles)]
    loads = np.zeros(n_tiles, np.int64)
    for node in order:
        while True:
            load, b = heapq.heappop(heap)
            if len(members[b]) < caps[b]:
                break
        members[b].append(node)
        loads[b] += deg[node]
        if len(members[b]) < caps[b]:
            heapq.heappush(heap, (int(loads[b]), b))
    perm_o2n = np.empty(n, np.int64)
    per_core = tiles_per_core * 128 - (128 - rows_last)
    for b in range(n_tiles):
        core, t = divmod(b, tiles_per_core)
        base = core * per_core + t * 128
        ids = np.asarray(members[b], np.int64)
        perm_o2n[ids] = base + np.arange(len(ids))
    return perm_o2n


def _wrap16_rep(a):
    """[L] int -> [128, L/16] int16 (16-wrap, replicated 8x down partitions)."""
    w = a.reshape(-1, 16).T.astype(np.int16)
    return np.ascontiguousarray(np.tile(w, (8, 1)))


def _host_prep(inputs, n_cores=N_CORES):
    import ml_dtypes

    bf16 = ml_dtypes.bfloat16
    x = np.asarray(inputs["x"], np.float32)
    ei = np.asarray(inputs["edge_index"])
    n, f_in = x.shape
    loop = np.arange(n, dtype=ei.dtype)
    src = np.concatenate([ei[0], loop]).astype(np.int64)
    dst = np.concatenate([ei[1], loop]).astype(np.int64)

    per_core = n // n_cores
    tiles_per_core = math.ceil(per_core / 128)
    rows_last = per_core - (tiles_per_core - 1) * 128

    perm = _balance_permutation(dst, n, n_cores, tiles_per_core, rows_last)

    # pfull row numbering: half-major when the AllGather is split in two
    # ([half0: all cores' tiles 0..T2-1][half1: rest], each half's
    # concatenated-by-core output contiguous), else plain core-major.
    T2 = tiles_per_core // 2
    hr = T2 * 128                      # half0 rows per core
    hr2 = per_core - hr
    local = perm % per_core
    core = perm // per_core
    if SPLIT_AG:
        pfull_row = np.where(local < hr,
                             core * hr + local,
                             n_cores * hr + core * hr2 + (local - hr))
    else:
        pfull_row = perm

    src_n = pfull_row[src]             # gather indices use pfull numbering
    dst_n = perm[dst]                  # dst partitioning uses (core, local)

    core_of = dst_n // per_core
    loc_of = dst_n % per_core
    counts = np.zeros((n_cores, tiles_per_core), np.int64)
    per_ct_src = {}
    per_ct_loc = {}
    for c in range(n_cores):
        sel = core_of == c
        s, loc = src_n[sel], loc_of[sel]
        o = np.argsort(loc, kind="stable")
        s, loc = s[o], loc[o]
        tile_of = loc // 128
        for t in range(tiles_per_core):
            m = tile_of == t
            per_ct_src[c, t] = s[m]
            per_ct_loc[c, t] = loc[m] - t * 128
            counts[c, t] = m.sum()

    nchunk = math.ceil(counts.max() / 128)
    group = min(GROUP, nchunk)
    nchunk = math.ceil(nchunk / group) * group

    cap = nchunk * 128
    T = tiles_per_core
    src16 = np.zeros((n_cores, T, 128, nchunk * 8), np.int16)
    ohs = np.zeros((n_cores, T, 128, nchunk * 128), bf16)
    ohTs = np.zeros((n_cores, T, 128, nchunk * 128), bf16)
    pp = np.arange(128)
    for c in range(n_cores):
        for t in range(T):
            e = counts[c, t]
            ps = np.zeros(cap, np.int64)
            pl = np.full(cap, -1, np.int64)
            ps[:e] = per_ct_src[c, t]
            pl[:e] = per_ct_loc[c, t]
            src16[c, t] = _wrap16_rep(ps)
            dl = pl.reshape(nchunk, 128)  # [cch, p]
            for cch in range(nchunk):
                m = dl[cch] >= 0
                j = dl[cch, m]
                ohs[c, t, pp[m], cch * 128 + j] = 1
                ohTs[c, t, j, cch * 128 + pp[m]] = 1

    # permuted node features, transposed, padded rows, bf16, per core
    rows_pad = T * 128
    x_perm = np.zeros((n, f_in), np.float32)
    x_perm[perm] = x
    xT = []
    for c in range(n_cores):
        blk = np.zeros((rows_pad, f_in), np.float32)
        blk[:per_core] = x_perm[c * per_core : (c + 1) * per_core]
        xT.append(np.ascontiguousarray(blk.T).astype(bf16))

    g = lambda k: np.asarray(inputs[k], np.float32)
    h1, c1 = g("a1s").shape
    h3, c3 = g("a3s").shape
    d1 = h1 * c1

    def fold(Wk, ak_s, ak_d, h, cc):
        W = g(Wk)  # [din, h*cc]
        a_s, a_d = g(ak_s), g(ak_d)  # [h, cc]
        waS = np.stack([W[:, i * cc : (i + 1) * cc] @ a_s[i] for i in range(h)], 1)
        waD = np.stack([W[:, i * cc : (i + 1) * cc] @ a_d[i] for i in range(h)], 1)
        return np.concatenate([waS, waD], 1).astype(bf16)  # [din, 2h]

    waug1 = np.concatenate([g("W1"), g("Wl1")], 1).astype(bf16)   # [50, 2048]
    waug2 = np.concatenate([g("W2"), g("Wl2")], 1).astype(bf16)   # [1024, 2048]
    waug3 = np.concatenate([g("W3"), g("Wl3")], 1).astype(bf16)   # [1024, 847]
    wsd1 = fold("W1", "a1s", "a1d", h1, c1)
    wsd2 = fold("W2", "a2s", "a2d", h1, c1)
    wsd3 = fold("W3", "a3s", "a3d", h3, c3)

    rep = lambda v: np.ascontiguousarray(
        np.broadcast_to(v[None, :], (128, v.shape[0]))
    ).astype(np.float32)
    base = dict(
        waug1=waug1, waug2=waug2, waug3=waug3,
        wsd1=wsd1, wsd2=wsd2, wsd3=wsd3,
        bsum1=rep(g("b1") + g("bl1")),
        bsum2=rep(g("b2") + g("bl2")),
        bsum3=rep(g("b3") + g("bl3")),
        idmat=np.eye(128, dtype=bf16),
    )
    in_maps = []
    for c in range(n_cores):
        m = dict(base)
        m["xT1"] = xT[c]
        m["src16"] = src16[c]
        m["oh"] = np.ascontiguousarray(ohs[c])
        m["ohT"] = np.ascontiguousarray(ohTs[c])
        in_maps.append(m)

    cfg = dict(
        n=n, f_in=f_in, n_cores=n_cores, per_core=per_core,
        tiles_per_core=T, rows_last=rows_last, rows_pad=rows_pad,
        nchunk=nchunk, group=group, t2=T2, hr=hr, hr2=hr2,
        h1=h1, c1=c1, d1=d1, h3=h3, c3=c3,
    )
    return in_maps, cfg, perm


# --------------------------------------------------------------------------
# bass program
# --------------------------------------------------------------------------

def _layer_dims(cfg):
    """Static per-layer dims. Payload row (bf16/u16 units):
    [h0 | 1 | h1 | 1 | ... ] (H*(C+1)=HST) then es (H f32), pad to 256B."""
    out = []
    for li in (1, 2, 3):
        if li < 3:
            h, c = cfg["h1"], cfg["c1"]
            din = cfg["f_in"] if li == 1 else cfg["d1"]
            nlin = cfg["d1"]
        else:
            h, c = cfg["h3"], cfg["c3"]
            din = cfg["d1"]
            nlin = cfg["c3"]
        st = c + 1
        hst = h * st
        es = hst // 2                      # f32 offset of es (hst is even)
        pw = math.ceil((hst + 2 * h) / 128) * 128   # u16 width, 256B multiple
        kch = math.ceil(din / 128)
        hc = h * c
        out.append(dict(li=li, din=din, kch=kch, hc=hc, nlin=nlin,
                        h=h, c=c, st=st, hst=hst, es=es, pw=pw,
                        naug=hc + nlin))
    return out


def _build(cfg):
    import concourse.bass as bass
    import concourse.bacc as bacc
    import concourse.mybir as mybir
    import concourse.tile as tile
    from contextlib import ExitStack

    f32 = mybir.dt.float32
    bf = mybir.dt.bfloat16
    i16 = mybir.dt.int16
    u16 = mybir.dt.uint16
    EXP = mybir.ActivationFunctionType.Exp
    CPY = mybir.ActivationFunctionType.Copy
    ALU = mybir.AluOpType

    n_cores = cfg["n_cores"]
    n = cfg["n"]
    T = cfg["tiles_per_core"]
    rows_last = cfg["rows_last"]
    per_core = cfg["per_core"]
    NCHUNK = cfg["nchunk"]
    GRP = cfg["group"]
    NG = NCHUNK // GRP
    D1 = cfg["d1"]
    layers = _layer_dims(cfg)

    nc = bacc.Bacc(None, target_bir_lowering=False, num_swdge_queues=2)

    # ---- parameters -----------------------------------------------------
    xT1 = nc.declare_dram_parameter("xT1", [cfg["f_in"], T * 128], bf, isOutput=False)
    waug_p, wsd_p, bsum_p = {}, {}, {}
    for L in layers:
        li = L["li"]
        waug_p[li] = nc.declare_dram_parameter(
            f"waug{li}", [L["din"], L["naug"]], bf, isOutput=False)
        wsd_p[li] = nc.declare_dram_parameter(
            f"wsd{li}", [L["din"], 2 * L["h"]], bf, isOutput=False)
        bsum_p[li] = nc.declare_dram_parameter(
            f"bsum{li}", [128, L["nlin"]], f32, isOutput=False)
    src16_p = nc.declare_dram_parameter("src16", [T, 128, NCHUNK * 8], i16, isOutput=False)
    oh_p = nc.declare_dram_parameter("oh", [T, 128, NCHUNK * 128], bf, isOutput=False)
    ohT_p = nc.declare_dram_parameter("ohT", [T, 128, NCHUNK * 128], bf, isOutput=False)
    id_p = nc.declare_dram_parameter("idmat", [128, 128], bf, isOutput=False)
    out_p = nc.declare_dram_parameter("out", [per_core, cfg["c3"]], f32, isOutput=True)
    dbg = {}
    if DEBUG_PROBES:
        L0 = layers[0]
        for nm, shape, dt_ in [
            ("dbg_pt", [128, L0["pw"]], u16),
            ("dbg_ed", [128, L0["h"]], bf),
            ("dbg_pe", [128, NCHUNK * L0["h"]], f32),
            ("dbg_tl", [128, GRP * L0["h"]], f32),
            ("dbg_gw", [128, L0["hst"]], u16),
            ("dbg_pa", [128, L0["hst"]], f32),
            ("dbg_xt", [128, L0["hc"]], f32),
            ("dbg_xo", [128, L0["hc"]], bf),
            ("dbg_xT", [128, L0["hc"]], bf),
        ]:
            dbg[nm] = nc.declare_dram_parameter(nm, shape, dt_, isOutput=True)

    with tile.TileContext(nc, num_cores=n_cores) as tc, ExitStack() as ctx:
        # ---- dram scratch ----------------------------------------------
        dram = ctx.enter_context(tc.tile_pool(name="dram", bufs=1, space="DRAM"))
        pshard = {L["li"]: dram.tile([per_core, L["pw"]], u16, tag=f"pshard{L['li']}",
                                     name=f"pshard{L['li']}") for L in layers}
        pfull = {L["li"]: dram.tile([n, L["pw"]], u16, tag=f"pfull{L['li']}",
                                    name=f"pfull{L['li']}", addr_space="Shared")
                 for L in layers}
        linb = {L["li"]: dram.tile([T * 128, L["nlin"]], f32, tag=f"lin{L['li']}",
                                   name=f"lin{L['li']}") for L in layers}

        # ---- pools ------------------------------------------------------
        consts = ctx.enter_context(tc.tile_pool(name="consts", bufs=1))
        wtp = ctx.enter_context(tc.tile_pool(name="wtp", bufs=1))
        xtp = ctx.enter_context(tc.tile_pool(name="xtp", bufs=2))
        xTp = ctx.enter_context(tc.tile_pool(name="xTp", bufs=2))
        ptp = ctx.enter_context(tc.tile_pool(name="ptp", bufs=2))
        ltp = ctx.enter_context(tc.tile_pool(name="ltp", bufs=2))
        gp = ctx.enter_context(tc.tile_pool(name="gp", bufs=3))
        idxp = ctx.enter_context(tc.tile_pool(name="idxp", bufs=2))
        ohp = ctx.enter_context(tc.tile_pool(name="ohp", bufs=2))
        lgp = ctx.enter_context(tc.tile_pool(name="lgp", bufs=3))
        epip = ctx.enter_context(tc.tile_pool(name="epip", bufs=1))
        recp = ctx.enter_context(tc.tile_pool(name="recp", bufs=4))
        psum_d = ctx.enter_context(tc.tile_pool(name="psum_d", bufs=1, space="PSUM"))
        psum_a = ctx.enter_context(tc.tile_pool(name="psum_a", bufs=1, space="PSUM"))
        psum_e = ctx.enter_context(tc.tile_pool(name="psum_e", bufs=2, space="PSUM"))
        psum_t = ctx.enter_context(tc.tile_pool(name="psum_t", bufs=1, space="PSUM"))

        # ---- constants ---------------------------------------------------
        idm = consts.tile([128, 128], bf, tag="idm")
        nc.sync.dma_start(out=idm[:, :], in_=id_p[:, :])
        wt = {}   # (li, k) -> weight tile [kk, naug]
        wsd = {}  # (li, k) -> [kk, 2H]
        bsum = {}
        edts = {}
        for L in layers:
            li, KCH, DIN = L["li"], L["kch"], L["din"]
            for k in range(KCH):
                kk = min(128, DIN - k * 128)
                w = wtp.tile([128, L["naug"]], bf, tag=f"w{li}_{k}", name=f"w{li}_{k}")
                nc.sync.dma_start(out=w[:kk, :], in_=waug_p[li][k * 128 : k * 128 + kk, :])
                wt[li, k] = w
                s = wtp.tile([128, 2 * L["h"]], bf, tag=f"s{li}_{k}", name=f"s{li}_{k}")
                nc.sync.dma_start(out=s[:kk, :], in_=wsd_p[li][k * 128 : k * 128 + kk, :])
                wsd[li, k] = s
            b = consts.tile([128, L["nlin"]], f32, tag=f"b{li}", name=f"b{li}")
            nc.sync.dma_start(out=b[:, :], in_=bsum_p[li][:, :])
            bsum[li] = b
            e = consts.tile([128, T * L["h"]], bf, tag=f"e{li}", name=f"e{li}")
            edts[li] = e

        HMAX = max(L["h"] for L in layers)
        ed_sb = consts.tile([128, T * NCHUNK * HMAX], f32, tag="edsb", name="ed_sb")

        def rows_of(t):
            return 128 if t < T - 1 else rows_last

        # ed per edge via ohT^T @ edt, for all tiles of a layer; no pfull
        # dependency, so this fills the AllGather window.
        def ed_preloop(L):
            li, H = L["li"], L["h"]
            for t in range(T):
                ohT_sb = ohp.tile([128, NCHUNK * 128], bf, tag="ohT")
                nc.sync.dma_start(out=ohT_sb[:, :], in_=ohT_p[t])
                pe = psum_e.tile([128, NCHUNK * H], f32, tag="pe", name="pe")
                for cch in range(NCHUNK):
                    nc.tensor.matmul(pe[:, cch * H : (cch + 1) * H],
                                     ohT_sb[:, cch * 128 : (cch + 1) * 128],
                                     edts[li][:, t * H : (t + 1) * H],
                                     start=True, stop=True)
                nc.scalar.activation(
                    ed_sb[:, t * NCHUNK * H : (t + 1) * NCHUNK * H], pe[:, :], CPY)

        def chunks(w):
            return [(c0, min(c0 + 512, w)) for c0 in range(0, w, 512)]

        # ---------------- dense phase for one tile -----------------------
        def dense_tile(L, t, get_lhsT):
            li, DIN, KCH = L["li"], L["din"], L["kch"]
            H, C, ST, HST, HC = L["h"], L["c"], L["st"], L["hst"], L["hc"]
            NLIN, ES, PW = L["nlin"], L["es"], L["pw"]
            r = rows_of(t)

            lhsTs = {}

            def lhsT_of(k, kk):
                if k not in lhsTs:
                    lhsTs[k] = get_lhsT(k, kk)
                return lhsTs[k]

            wA = HC + NLIN if li == 3 else HC
            pse = psum_e.tile([128, 2 * H], f32, tag="pe", name="pse")
            pdA = psum_d.tile([128, 1024], f32, tag="pd", name="pdA")
            for k in range(KCH):
                kk = min(128, DIN - k * 128)
                lhsT = lhsT_of(k, kk)
                st, sp = (k == 0), (k == KCH - 1)
                for c0, c1 in chunks(wA):
                    nc.tensor.matmul(pdA[:, c0:c1], lhsT[:kk, :],
                                     wt[li, k][:kk, c0:c1], start=st, stop=sp)
                nc.tensor.matmul(pse[:, :], lhsT[:kk, :], wsd[li, k][:kk, :],
                                 start=st, stop=sp)

            # payload assembly
            pt = ptp.tile([128, PW], u16, tag="pt")
            ptb = pt.bitcast(bf)
            for h in range(H):
                nc.scalar.activation(ptb[:, h * ST : h * ST + C],
                                     pdA[:, h * C : (h + 1) * C], CPY)
            ones_v = ptb[:, :HST].rearrange("p (h s) -> p h s", h=H)[:, :, C : C + 1]
            nc.vector.memset(ones_v, 1.0)
            ptf = pt.bitcast(f32)
            nc.scalar.activation(ptf[:, ES : ES + H], pse[:, :H], CPY)
            nc.vector.memset(pt[:, 2 * (ES + H) : PW], 0)
            nc.scalar.activation(edts[li][:, t * H : (t + 1) * H], pse[:, H : 2 * H], CPY)
            nc.sync.dma_start(out=pshard[li][t * 128 : t * 128 + r, :], in_=pt[:r, :])
            # lin + bias staging (second pass reuses the pd psum slot for li<3)
            if li < 3:
                pdB = psum_d.tile([128, 1024], f32, tag="pd", name="pdB")
                for k in range(KCH):
                    kk = min(128, DIN - k * 128)
                    lhsT = lhsT_of(k, kk)
                    st, sp = (k == 0), (k == KCH - 1)
                    for c0, c1 in chunks(NLIN):
                        nc.tensor.matmul(pdB[:, c0:c1], lhsT[:kk, :],
                                         wt[li, k][:kk, HC + c0 : HC + c1],
                                         start=st, stop=sp)
            else:
                pdB = pdA
            loff = HC if li == 3 else 0
            lt = ltp.tile([128, NLIN], f32, tag="lt")
            nc.vector.tensor_tensor(out=lt[:, :], in0=pdB[:, loff : loff + NLIN],
                                    in1=bsum[li][:, :], op=ALU.add)
            nc.sync.dma_start(out=linb[li][t * 128 : t * 128 + r, :], in_=lt[:r, :])
            if DEBUG_PROBES and li == 1 and t == 0:
                nc.sync.dma_start(out=dbg["dbg_pt"][:, :], in_=pt[:, :])
                nc.sync.dma_start(out=dbg["dbg_ed"][:, :], in_=edts[li][:, 0:H])

        # ---------------- aggregation for one tile ------------------------
        def agg_tile(L, t):
            li = L["li"]
            H, C, ST, HST, HC = L["h"], L["c"], L["st"], L["hst"], L["hc"]
            NLIN, ES, PW = L["nlin"], L["es"], L["pw"]
            PWF = PW // 2
            r = rows_of(t)

            s16 = idxp.tile([128, NCHUNK * 8], i16, tag="s16")
            nc.sync.dma_start(out=s16[:, :], in_=src16_p[t])
            oh_sb = ohp.tile([128, NCHUNK * 128], bf, tag="oh")
            nc.sync.dma_start(out=oh_sb[:, :], in_=oh_p[t])

            pa = psum_a.tile([128, HST], f32, tag="pa", name="pa")
            for g in range(NG):
                G = gp.tile([128, GRP, PW], u16, tag="G")
                nc.gpsimd.dma_gather(
                    out_ap=G[:, :, :],
                    in_ap=pfull[li][:, :],
                    idxs_ap=s16[:, g * GRP * 8 : (g + 1) * GRP * 8],
                    num_idxs=GRP * 128,
                    num_idxs_reg=GRP * 128,
                    elem_size=PW,
                    queue_num=(t * NG + g) % 2,
                )
                Gf = G.bitcast(f32)
                Gb = G.bitcast(bf)
                tl = lgp.tile([128, GRP, H], f32, tag="tl")
                wf = lgp.tile([128, GRP, H], f32, tag="wf")
                nc.vector.tensor_tensor(
                    out=tl[:, :, :], in0=Gf[:, :, ES : ES + H],
                    in1=ed_sb[:, (t * NCHUNK + g * GRP) * H :
                              (t * NCHUNK + (g + 1) * GRP) * H].rearrange(
                        "p (g h) -> p g h", h=H),
                    op=ALU.add,
                )
                nc.vector.scalar_tensor_tensor(
                    out=wf[:, :, :], in0=tl[:, :, :], scalar=0.2, in1=tl[:, :, :],
                    op0=ALU.mult, op1=ALU.max,
                )
                web = lgp.tile([128, GRP, H], bf, tag="web")
                nc.scalar.activation(web[:, :, :], wf[:, :, :], EXP)
                Gh = Gb[:, :, :HST].rearrange("p g (h s) -> p g h s", h=H)
                webB = web.unsqueeze(3).broadcast_to([128, GRP, H, ST])
                nc.vector.tensor_tensor(out=Gh, in0=Gh, in1=webB, op=ALU.mult)
                Gw = Gb[:, :, :HST]
                if DEBUG_PROBES and li == 1 and t == 0 and g == 0:
                    nc.sync.dma_start(out=dbg["dbg_pe"][:, :],
                                      in_=ed_sb[:, : NCHUNK * H])
                    nc.sync.dma_start(out=dbg["dbg_tl"][:, :],
                                      in_=tl.rearrange("p g h -> p (g h)"))
                    nc.sync.dma_start(out=dbg["dbg_gw"][:, :], in_=G[:, 0, :HST])
                for cch in range(GRP):
                    j = g * GRP + cch
                    for c0, c1 in chunks(HST):
                        nc.tensor.matmul(
                            pa[:, c0:c1],
                            oh_sb[:, j * 128 : (j + 1) * 128],
                            Gw[:, cch, c0:c1],
                            start=(j == 0),
                            stop=(j == NCHUNK - 1),
                        )

            # ---- epilogue ----
            xt = epip.tile([128, HC], f32, tag="xt")
            for h in range(H):
                dn = recp.tile([128, 1], f32, tag="dn")
                nc.vector.tensor_scalar(out=dn[:, :], in0=pa[:, h * ST + C : h * ST + C + 1],
                                        scalar1=1e-30, scalar2=None, op0=ALU.max)
                rec = recp.tile([128, 1], f32, tag="rec")
                nc.vector.reciprocal(rec[:, :], dn[:, :])
                nc.vector.tensor_scalar(
                    out=xt[:, h * C : (h + 1) * C], in0=pa[:, h * ST : h * ST + C],
                    scalar1=rec[:, 0:1], scalar2=None, op0=ALU.mult,
                )
            if DEBUG_PROBES and li == 1 and t == 0:
                pac = epip.tile([128, HST], f32, tag="pac")
                nc.vector.tensor_copy(pac[:, :], pa[:, :])
                nc.sync.dma_start(out=dbg["dbg_pa"][:, :], in_=pac[:, :])
                nc.sync.dma_start(out=dbg["dbg_xt"][:, :], in_=xt[:, :])
            lt2 = ltp.tile([128, NLIN], f32, tag="lt2")
            nc.sync.dma_start(out=lt2[:r, :], in_=linb[li][t * 128 : t * 128 + r, :])
            if li < 3:
                u = epip.tile([128, HC], f32, tag="u")
                e = epip.tile([128, HC], f32, tag="e")
                xo = epip.tile([128, HC], bf, tag="xo")
                if r < 128:
                    nc.vector.memset(xo[:, :], 0)
                nc.vector.tensor_tensor(out=xt[:r, :], in0=xt[:r, :], in1=lt2[:r, :],
                                        op=ALU.add)
                nc.vector.tensor_scalar(out=u[:r, :], in0=xt[:r, :], scalar1=0.0,
                                        scalar2=None, op0=ALU.min)
                nc.scalar.activation(e[:r, :], u[:r, :], EXP)
                nc.vector.tensor_scalar(out=xt[:r, :], in0=xt[:r, :], scalar1=0.0,
                                        scalar2=-1.0, op0=ALU.max, op1=ALU.add)
                nc.vector.tensor_tensor(out=xo[:r, :], in0=xt[:r, :], in1=e[:r, :],
                                        op=ALU.add)
                # PE transpose -> next layer lhsT in SBUF
                ptr = psum_t.tile([128, HC], bf, tag="ptr", name="ptr")
                for k in range(HC // 128):
                    nc.tensor.transpose(ptr[:, k * 128 : (k + 1) * 128],
                                        xo[:, k * 128 : (k + 1) * 128], idm[:, :])
                xT_sb = xTp.tile([128, HC], bf, tag="xT")
                nc.scalar.activation(xT_sb[:, :], ptr[:, :], CPY)
                if DEBUG_PROBES and li == 1 and t == 0:
                    nc.sync.dma_start(out=dbg["dbg_xo"][:, :], in_=xo[:, :])
                    nc.sync.dma_start(out=dbg["dbg_xT"][:, :], in_=xT_sb[:, :])
                return xT_sb
            else:
                xt3 = xt.rearrange("p (h c) -> p h c", h=H)
                hh = H // 2
                m1 = epip.tile([128, hh, C], f32, tag="m1")
                nc.vector.tensor_tensor(out=m1[:, :, :], in0=xt3[:, 0:hh, :],
                                        in1=xt3[:, hh : 2 * hh, :], op=ALU.add)
                m2 = epip.tile([128, C], f32, tag="m2")
                nc.vector.tensor_tensor(out=m2[:, :], in0=m1[:, 0, :], in1=m1[:, 1, :],
                                        op=ALU.add)
                for i in range(2, hh):
                    nc.vector.tensor_tensor(out=m2[:, :], in0=m2[:, :], in1=m1[:, i, :],
                                            op=ALU.add)
                ot = epip.tile([128, C], f32, tag="ot")
                nc.vector.scalar_tensor_tensor(
                    out=ot[:r, :], in0=m2[:r, :], scalar=1.0 / H, in1=lt2[:r, :],
                    op0=ALU.mult, op1=ALU.add,
                )
                nc.sync.dma_start(out=out_p[t * 128 : t * 128 + r, :], in_=ot[:r, :])
                return None

        # ------------------------------------------------------------------
        T2 = cfg["t2"]
        HR = cfg["hr"]

        def allgather(li, half):
            # half-major global numbering keeps both AG outputs contiguous
            if half == 0:
                ins_ap = pshard[li][0:HR, :]
                outs_ap = pfull[li][0 : n_cores * HR, :]
            else:
                ins_ap = pshard[li][HR:per_core, :]
                outs_ap = pfull[li][n_cores * HR : n, :]
            nc.gpsimd.collective_compute(
                "AllGather",
                ALU.bypass,
                replica_groups=[list(range(n_cores))],
                ins=[ins_ap.opt()],
                outs=[outs_ap.opt()],
            )

        def maybe_ag(li, t):
            if SPLIT_AG:
                if t == T2 - 1:
                    allgather(li, 0)
                elif t == T - 1:
                    allgather(li, 1)
            elif t == T - 1:
                nc.gpsimd.collective_compute(
                    "AllGather",
                    ALU.bypass,
                    replica_groups=[list(range(n_cores))],
                    ins=[pshard[li].opt()],
                    outs=[pfull[li].opt()],
                )

        # ---- layer 1 dense (lhsT from host-prepped xT1) -------------------
        def l1_lhsT_for(t):
            def get(k, kk):
                lhsT = xtp.tile([128, 128], bf, tag="lhsT", name="lhsT")
                nc.sync.dma_start(out=lhsT[:kk, :], in_=xT1[:, t * 128 : (t + 1) * 128])
                return lhsT
            return get

        for t in range(T):
            dense_tile(layers[0], t, l1_lhsT_for(t))
            maybe_ag(1, t)

        # ---- agg L1 + dense L2, agg L2 + dense L3, agg L3 -----------------
        ed_preloop(layers[0])
        for t in range(T):
            xT_sb = agg_tile(layers[0], t)
            dense_tile(layers[1], t, lambda k, kk, x=xT_sb: x[:, k * 128 : (k + 1) * 128])
            maybe_ag(2, t)
        ed_preloop(layers[1])
        for t in range(T):
            xT_sb = agg_tile(layers[1], t)
            dense_tile(layers[2], t, lambda k, kk, x=xT_sb: x[:, k * 128 : (k + 1) * 128])
            maybe_ag(3, t)
        ed_preloop(layers[2])
        for t in range(T):
            agg_tile(layers[2], t)

    nc.finalize()
    return nc


# --------------------------------------------------------------------------
# runner
# --------------------------------------------------------------------------

def _run(inputs, sim=False, trace=False, n_cores=N_CORES, tmpdir=None):
    in_maps, cfg, perm = _host_prep(inputs, n_cores)
    nc = _build(cfg)
    if sim:
        import concourse.bass_interp as bass_interp

        msim = bass_interp.MultiCoreSim(nc, n_cores)
        for c in range(n_cores):
            for k, v in in_maps[c].items():
                msim.cores[c].tensor(k)[:] = v
        msim.simulate(check_with_hw=True)
        outs = [np.array(msim.cores[c].mem_tensor("out")) for c in range(n_cores)]
        exec_ns = None
    else:
        from concourse.bass_utils import run_bass_kernel_spmd

        res = run_bass_kernel_spmd(
            nc, in_maps, list(range(n_cores)), trace=trace, tmpdir=tmpdir
        )
        outs = [res.results[c]["out"] for c in range(n_cores)]
        exec_ns = res.exec_time_ns
    out_new = np.concatenate(outs, 0)       # rows in (core, local) order
    out = np.empty_like(out_new)
    out[...] = out_new[perm]
    return out.astype(np.float32), exec_ns


def kernel(**inputs) -> np.ndarray:
    out, _ = _run(inputs)
    return out
